# revision 3
# baseline (speedup 1.0000x reference)
"""LIF spiking-neuron kernel (nn_Neuron_75222057222206) for 8x TRN2 NeuronCores.

Reference semantics (per timestep t, elementwise over [B, N] state):
    u = tau_c * u + x[:, t]        (leaky integration, tau_c = clip(tau,0,1))
    o = (u - 1.0 > 0).float()      (spike)
    u = u * (1.0 - o)              (multiplicative reset)
Output: o stacked over t -> [B, T, N] float32.

Sharding: pure data-parallel over batch. B=32 -> 4 batch rows per core,
zero communication. Per-core state u is [4, 65536] f32 = 1 MB, held in
SBUF as [128 partitions x 2048], (b, n) -> partition b*32 + n//2048,
free n%2048.

Per timestep per core: DMA in 1 MB of x, 3 DVE ops, DMA out 1 MB of o.
tau is baked in at trace time as an immediate (it is a scalar input;
the kernel is compiled per call, so this is just compile-time constant
specialization - any tau value works).
"""

import numpy as np

B, T, N = 32, 32, 65536
NCORES = 8
BL = B // NCORES          # batch rows per core (4)
P = 128                   # SBUF partitions
F = (BL * N) // P         # free elements per partition (2048)
QP = N // F               # partitions per batch row (32)
THRESH = 1.0

# test.py may flip this to get an NTFF profile + exec time out of the run.
TRACE = False
LAST_RESULTS = None       # stash of BassKernelResults when TRACE


def _ensure_import_path():
    import sys
    try:
        import concourse  # noqa: F401
    except ImportError:
        sys.path.insert(0, "/opt/trn_rl_repo")


def build(nc, tau_c: float):
    """Emit the per-core LIF kernel into Bass object `nc`."""
    import concourse.mybir as mybir
    import concourse.tile as tile

    f32 = mybir.dt.float32
    Alu = mybir.AluOpType

    x_d = nc.dram_tensor("x", [BL, T, N], f32, kind="ExternalInput")
    o_d = nc.dram_tensor("o", [BL, T, N], f32, kind="ExternalOutput")

    # [BL, T, N] -> [T, BL, QP, F]; per-t slice is the DRAM side of a
    # [128, F] SBUF tile (partition dim = (b, q)).
    x_r = x_d.ap().rearrange("b t (q f) -> t b q f", f=F)
    o_r = o_d.ap().rearrange("b t (q f) -> t b q f", f=F)

    with tile.TileContext(nc) as tc:
        with (
            tc.tile_pool(name="xp", bufs=4) as xp,
            tc.tile_pool(name="op", bufs=4) as op,
            tc.tile_pool(name="vp", bufs=2) as vp,
            tc.tile_pool(name="up", bufs=2) as up,
        ):
            u = None
            for t in range(T):
                xt = xp.tile([P, F], f32)
                nc.sync.dma_start(xt[:], x_r[t])
                if t == 0:
                    v = xt  # u0 == 0 so v = x[0]
                else:
                    v = vp.tile([P, F], f32)
                    nc.vector.scalar_tensor_tensor(
                        v[:], u[:], tau_c, xt[:], Alu.mult, Alu.add
                    )
                ot = op.tile([P, F], f32)
                nc.vector.tensor_scalar(ot[:], v[:], THRESH, None, Alu.is_gt)
                u2 = up.tile([P, F], f32)
                nc.vector.scalar_tensor_tensor(
                    u2[:], v[:], THRESH, v[:], Alu.is_le, Alu.mult
                )
                u = u2
                nc.sync.dma_start(o_r[t], ot[:])
    return x_d, o_d


def make_nc(tau_c: float):
    _ensure_import_path()
    from concourse import bacc

    nc = bacc.Bacc("TRN2", target_bir_lowering=False, debug=False)
    build(nc, tau_c)
    nc.compile()
    return nc


def kernel(x, tau):
    global LAST_RESULTS
    _ensure_import_path()
    from concourse.bass_utils import run_bass_kernel_spmd

    x = np.ascontiguousarray(np.asarray(x, dtype=np.float32))
    tau_c = float(np.clip(np.asarray(tau, dtype=np.float32), 0.0, 1.0).ravel()[0])
    assert x.shape == (B, T, N), x.shape

    nc = make_nc(tau_c)
    in_maps = [{"x": x[c * BL : (c + 1) * BL]} for c in range(NCORES)]
    res = run_bass_kernel_spmd(nc, in_maps, list(range(NCORES)), trace=TRACE)
    LAST_RESULTS = res
    out = np.concatenate([res.results[c]["o"] for c in range(NCORES)], axis=0)
    return out
